# revision 2
# baseline (speedup 1.0000x reference)
"""ChameleonAttention Trainium2 kernel.

Full-input contract: kernel(**inputs) with the complete tensors; internally
shards tensor-parallel across 8 NeuronCores by attention head (4 heads/core):
  - w_qkv rows + q/k norm params sharded by head
  - w_o columns sharded by head, partial outputs summed on host (all-reduce)

Per-core dataflow (matmuls in fp32r, 1 cycle/row):
  P1: qkvT[r, t] = w_qkv_c @ hidden^T via PE-transposed tiles; K split in two
      halves (wT half kept SBUF-resident), accumulated into a DRAM scratch
      with an accumulate-DMA on the second half.
  P2: per head: LayerNorm (PE ones-matmul stats + K=1 broadcast matmuls),
      neox RoPE (device-computed sin/cos with Cody-Waite range reduction),
      causal attention streamed over key tiles in S^T layout: softmax
      denominator via ones-matmul (no max subtraction; scores are O(1)),
      E = exp(scale*S^T) straight out of PSUM, P@V accumulated transposed.
  P3: o_proj: out[t, :] = attnT^T @ w_o_c^T with PE-transposed w_o tiles.
"""
import sys

sys.path.insert(0, "/opt/trn_rl_repo")

import numpy as np

import concourse.bass as bass
import concourse.mybir as mybir
import concourse.tile as tile
from concourse import bacc
from concourse.bass_utils import run_bass_kernel_spmd
from concourse.masks import make_identity, make_upper_triangular

P = 128
T = 2048
HID = 4096
D = 128
H_PER_CORE = 4
R = 3 * H_PER_CORE  # qkv row-tiles per core (4 q + 4 k + 4 v)
KH = HID // 2  # contraction half
TC = 256  # P1 t-chunk
THETA = 10000.0
EPS = 1e-5
SCALE = D ** -0.5
TWO_PI = 6.283185307179586
C_HI = float(np.float32(6.28125))
C_LO = TWO_PI - C_HI

f32 = mybir.dt.float32
f32r = mybir.dt.float32r
i32 = mybir.dt.int32
AF = mybir.ActivationFunctionType
ALU = mybir.AluOpType

_NC_CACHE = {}


def build_nc(n_iters: int = 1, dump: bool = False, phases: str = "123"):
    nc = bacc.Bacc(None, target_bir_lowering=False, debug=False)

    hidden = nc.dram_tensor("hidden", (T, HID), f32, kind="ExternalInput")
    wq = nc.dram_tensor("wq", (R * P, HID), f32, kind="ExternalInput")
    wo = nc.dram_tensor("wo", (HID, H_PER_CORE * D), f32, kind="ExternalInput")
    pos = nc.dram_tensor("pos", (T,), i32, kind="ExternalInput")
    qnw = nc.dram_tensor("qnw", (H_PER_CORE, D), f32, kind="ExternalInput")
    qnb = nc.dram_tensor("qnb", (H_PER_CORE, D), f32, kind="ExternalInput")
    knw = nc.dram_tensor("knw", (H_PER_CORE, D), f32, kind="ExternalInput")
    knb = nc.dram_tensor("knb", (H_PER_CORE, D), f32, kind="ExternalInput")
    out = nc.dram_tensor("out", (T, HID), f32, kind="ExternalOutput")

    if "1" in phases:
        qkvT = nc.dram_tensor("qkvT_scr", (R, P, T), f32)  # internal scratch
    else:
        qkvT = nc.dram_tensor("qkvT_scr", (R, P, T), f32, kind="ExternalInput")
    if "3" in phases and "2" not in phases:
        attnT_in = nc.dram_tensor("attnT_in", (P, H_PER_CORE, T), f32,
                                  kind="ExternalInput")
    if dump:
        d_qkvT = nc.dram_tensor("d_qkvT", (R, P, T), f32, kind="ExternalOutput")
        d_x = nc.dram_tensor("d_x", (8, P, T), f32, kind="ExternalOutput")
        d_attnT = nc.dram_tensor("d_attnT", (P, H_PER_CORE, T), f32, kind="ExternalOutput")

    with tile.TileContext(nc) as tc:
        with tc.tile_pool(name="const", bufs=1) as const:
            # --- constants ---
            ident = const.tile([P, P], f32r)
            triu = const.tile([P, P], f32r)
            ones_c = const.tile([P, 1], f32r)
            ones_r = const.tile([1, P], f32r)
            zeros_r = const.tile([P, 3 * P], f32r)
            eps8 = const.tile([8, 1], f32)
            nc.vector.memset(eps8[:], EPS)
            with tc.tile_pool(name="cstage", bufs=1) as cstage:
                ident_f = cstage.tile([P, P], f32)
                make_identity(nc, ident_f[:])
                nc.vector.tensor_copy(ident[:], ident_f[:])
                triu_f = cstage.tile([P, P], f32)
                make_upper_triangular(nc, triu_f[:], val=1.0, diag=True)
                nc.vector.tensor_copy(triu[:], triu_f[:])
                ones_f = cstage.tile([P, 1], f32)
                nc.vector.memset(ones_f[:], 1.0)
                nc.vector.tensor_copy(ones_c[:], ones_f[:])
                ones_rf = cstage.tile([1, P], f32)
                nc.vector.memset(ones_rf[:], 1.0)
                nc.vector.tensor_copy(ones_r[:], ones_rf[:])
                zeros_f = cstage.tile([P, 3 * P], f32)
                nc.vector.memset(zeros_f[:], 0.0)
                nc.vector.tensor_copy(zeros_r[:], zeros_f[:])

            # norm params as [128, 1] per-partition columns; cols = head
            nwq = const.tile([P, H_PER_CORE], f32)
            nbq = const.tile([P, H_PER_CORE], f32)
            nwk = const.tile([P, H_PER_CORE], f32)
            nbk = const.tile([P, H_PER_CORE], f32)
            for h in range(H_PER_CORE):
                for dst, src in ((nwq, qnw), (nbq, qnb), (nwk, knw), (nbk, knb)):
                    nc.sync.dma_start(
                        dst[:, h : h + 1], src[h : h + 1, :].rearrange("a b -> b a")
                    )

            # --- rope tables: c128 = [cos; cos], s128 = [-sin; sin] ---
            c128 = const.tile([P, T], f32)
            s128 = const.tile([P, T], f32)
            with tc.tile_pool(name="rope_tmp", bufs=1) as rtmp:
                tp_i = rtmp.tile([1, T], i32)
                nc.sync.dma_start(tp_i[:], pos[None, :])
                tp_f = rtmp.tile([1, T], f32)
                nc.vector.tensor_copy(tp_f[:], tp_i[:])
                posb = rtmp.tile([64, T], f32)
                nc.gpsimd.partition_broadcast(posb[:], tp_f[:], channels=64)
                jj = rtmp.tile([64, 1], f32)
                nc.gpsimd.iota(jj[:], pattern=[[1, 1]], base=0, channel_multiplier=1,
                               allow_small_or_imprecise_dtypes=True)
                invf = rtmp.tile([64, 1], f32)
                nc.scalar.activation(invf[:], jj[:], AF.Exp,
                                     scale=-float(np.log(THETA)) / 64.0)
                freqs = rtmp.tile([64, T], f32)
                nc.vector.tensor_scalar_mul(freqs[:], posb[:], invf[:])

                def reduced_sin(dst_ap, src_ap, sign):
                    # dst = sin(sign * reduce(src)), reduce(x) = x - 2pi*round(x/2pi)
                    q = rtmp.tile([64, T], f32, tag="rs_q")
                    nc.vector.tensor_scalar_mul(q[:], src_ap, 1.0 / TWO_PI)
                    n_i = rtmp.tile([64, T], i32, tag="rs_n")
                    nc.vector.tensor_copy(n_i[:], q[:])  # round-to-nearest
                    n_f = rtmp.tile([64, T], f32, tag="rs_nf")
                    nc.vector.tensor_copy(n_f[:], n_i[:])
                    r0 = rtmp.tile([64, T], f32, tag="rs_r0")
                    nc.vector.scalar_tensor_tensor(
                        out=r0[:], in0=n_f[:], scalar=-C_HI, in1=src_ap,
                        op0=ALU.mult, op1=ALU.add)
                    r1 = rtmp.tile([64, T], f32, tag="rs_r1")
                    nc.vector.scalar_tensor_tensor(
                        out=r1[:], in0=n_f[:], scalar=-C_LO, in1=r0[:],
                        op0=ALU.mult, op1=ALU.add)
                    nc.scalar.activation(dst_ap, r1[:], AF.Sin, scale=sign)

                reduced_sin(s128[0:64, :], freqs[:], -1.0)
                reduced_sin(s128[64:P, :], freqs[:], 1.0)
                fr2 = rtmp.tile([64, T], f32)
                nc.vector.tensor_scalar_add(fr2[:], freqs[:], np.pi / 2)
                reduced_sin(c128[0:64, :], fr2[:], 1.0)
                nc.vector.tensor_copy(c128[64:P, :], c128[0:64, :])

            def _phases(_iv=None):
                if "1" in phases:
                    # ---------------- P1: qkvT = wq @ hidden^T ----------------
                    with (
                        tc.tile_pool(name="p1_wld", bufs=2) as p_wld,
                        tc.tile_pool(name="p1_wT", bufs=1) as p_wT,
                        tc.tile_pool(name="p1_hld", bufs=2) as p_hld,
                        tc.tile_pool(name="p1_hT", bufs=2) as p_hT,
                        tc.tile_pool(name="p1_ev", bufs=2) as p_ev,
                        tc.tile_pool(name="p1_tps", bufs=2, space="PSUM") as p_tps,
                        tc.tile_pool(name="p1_pack", bufs=2, space="PSUM") as p_pack,
                    ):
                        NKK = KH // P  # 16 k-tiles per half
                        for kh in range(2):
                            k0 = kh * KH
                            wTs = [p_wT.tile([P, NKK, P], f32r, tag=f"wT{rt}", name=f"wT{rt}")
                                   for rt in range(R)]
                            for rt in range(R):
                                wld = p_wld.tile([P, KH], f32r, tag="wld")
                                nc.sync.dma_start(
                                    wld[:], wq[rt * P : (rt + 1) * P, k0 : k0 + KH].bitcast(f32r)
                                )
                                for kg in range(NKK // 4):
                                    pw = p_tps.tile([P, 4, P], f32r, tag="tps")
                                    for j in range(4):
                                        kk = kg * 4 + j
                                        nc.tensor.transpose(
                                            pw[:, j, :],
                                            wld[:, kk * P : (kk + 1) * P],
                                            ident[:],
                                        )
                                    nc.any.tensor_copy(
                                        wTs[rt][:, kg * 4 : kg * 4 + 4, :], pw[:])
                            for tc_i in range(T // TC):
                                t0 = tc_i * TC
                                hT = p_hT.tile([P, NKK, TC], f32r, tag="hT")
                                for tt in range(TC // P):
                                    hld = p_hld.tile([P, KH], f32r, tag="hld")
                                    nc.sync.dma_start(
                                        hld[:],
                                        hidden[t0 + tt * P : t0 + (tt + 1) * P,
                                               k0 : k0 + KH].bitcast(f32r),
                                    )
                                    for kg in range(NKK // 4):
                                        ph = p_tps.tile([P, 4, P], f32r, tag="tps")
                                        for j in range(4):
                                            kk = kg * 4 + j
                                            nc.tensor.transpose(
                                                ph[:, j, :],
                                                hld[:, kk * P : (kk + 1) * P],
                                                ident[:],
                                            )
                                        nc.any.tensor_copy(
                                            hT[:, kg * 4 : kg * 4 + 4, tt * P : (tt + 1) * P],
                                            ph[:],
                                        )
                                for rg in range(2):
                                    pack = p_pack.tile([P, 6, TC], f32, tag="pack")
                                    # rr outer / kk inner: each slot's accumulation chain
                                    # completes before the next one's start=True clears
                                    # the shared bank's has_written bits
                                    for rr in range(6):
                                        rt = rg * 6 + rr
                                        for kk in range(NKK):
                                            nc.tensor.matmul(
                                                pack[:, rr, :],
                                                wTs[rt][:, kk, :],
                                                hT[:, kk, :],
                                                start=(kk == 0),
                                                stop=(kk == NKK - 1),
                                            )
                                    ev = p_ev.tile([P, 6, TC], f32, tag="ev")
                                    nc.any.tensor_copy(ev[:, 0:3, :], pack[:, 0:3, :])
                                    nc.any.tensor_copy(ev[:, 3:6, :], pack[:, 3:6, :])
                                    dst = qkvT[rg * 6 : rg * 6 + 6, :, t0 : t0 + TC]
                                    dst = dst.rearrange("r p t -> p r t")
                                    if kh == 0:
                                        nc.sync.dma_start(dst, ev[:])
                                    else:
                                        nc.gpsimd.dma_start(dst, ev[:], accum_op=ALU.add)

                    if dump:
                        for rt in range(R):
                            nc.sync.dma_start(d_qkvT[rt], qkvT[rt])

                # ---------------- P2 + P3 ----------------
                with (
                    tc.tile_pool(name="p2_attnT", bufs=1) as p_attnT,
                    tc.tile_pool(name="ps_misc", bufs=2, space="PSUM") as ps_misc,
                ):
                    attnT = p_attnT.tile([P, H_PER_CORE, T], f32r, tag="attnT")

                    if "2" in phases:
                        with (
                            tc.tile_pool(name="p2_x", bufs=1) as p_x,
                            tc.tile_pool(name="p2_sq", bufs=1) as p_sq,
                            tc.tile_pool(name="p2_st", bufs=1) as p_st,
                            tc.tile_pool(name="p2_v", bufs=2) as p_v,
                            tc.tile_pool(name="p2_vsb", bufs=1) as p_vsb,
                            tc.tile_pool(name="p2_E", bufs=3) as p_E,
                            tc.tile_pool(name="p2_sw", bufs=1) as p_sw,
                            tc.tile_pool(name="p2_rec", bufs=2) as p_rec,
                        ):
                            xs = []
                            with tc.tile_pool(name="ps_stats", bufs=2, space="PSUM") as ps_stats:
                                for hh in range(2 * H_PER_CORE):
                                    x = p_x.tile([P, T], f32r, tag=f"x{hh}")
                                    xs.append(x)
                                    nc.sync.dma_start(x[:], qkvT[hh, :, :].bitcast(f32r))
                                    sq = p_sq.tile([P, T], f32r, tag="sq")
                                    nc.scalar.activation(sq[:], x[:], AF.Square)
                                    w_col = (nwq if hh < 4 else nwk)[:, (hh % 4) : (hh % 4) + 1]
                                    b_col = (nbq if hh < 4 else nbk)[:, (hh % 4) : (hh % 4) + 1]
                                    for c4 in range(4):
                                        cs = slice(c4 * 512, (c4 + 1) * 512)
                                        s1_ps = ps_stats.tile([1, 512], f32, tag="s1")
                                        nc.tensor.matmul(s1_ps[:], ones_c[:], x[:, cs],
                                                         start=True, stop=True)
                                        s2_ps = ps_stats.tile([1, 512], f32, tag="s2")
                                        nc.tensor.matmul(s2_ps[:], ones_c[:], sq[:, cs],
                                                         start=True, stop=True)
                                        mu_sb = p_st.tile([1, 512], f32r, tag="mu")
                                        nc.scalar.activation(mu_sb[:], s1_ps[:], AF.Copy,
                                                             scale=1.0 / D)
                                        musq = p_st.tile([1, 512], f32, tag="musq")
                                        nc.scalar.activation(musq[:], mu_sb[:], AF.Square)
                                        varv = p_st.tile([1, 512], f32, tag="varv")
                                        nc.vector.scalar_tensor_tensor(
                                            out=varv[:], in0=s2_ps[:], scalar=1.0 / D,
                                            in1=musq[:], op0=ALU.mult, op1=ALU.subtract)
                                        stdv = p_st.tile([1, 512], f32, tag="stdv")
                                        nc.scalar.activation(stdv[:], varv[:], AF.Sqrt,
                                                             bias=eps8[0:1, :])
                                        rstd_sb = p_st.tile([1, 512], f32r, tag="rstd")
                                        with nc.allow_low_precision(reason="f32r LN rstd"):
                                            nc.vector.reciprocal(rstd_sb[:], stdv[:])
                                        mu_b = ps_misc.tile([P, 512], f32, tag="misc")
                                        nc.tensor.matmul(mu_b[:], ones_r[:], mu_sb[:],
                                                         start=True, stop=True)
                                        rs_b = ps_misc.tile([P, 512], f32, tag="misc")
                                        nc.tensor.matmul(rs_b[:], ones_r[:], rstd_sb[:],
                                                         start=True, stop=True)
                                        nc.vector.tensor_sub(x[:, cs], x[:, cs], mu_b[:])
                                        nc.vector.tensor_mul(x[:, cs], x[:, cs], rs_b[:])
                                        nc.vector.tensor_scalar(
                                            out=x[:, cs], in0=x[:, cs],
                                            scalar1=w_col, scalar2=b_col,
                                            op0=ALU.mult, op1=ALU.add)

                            # rope per head
                            for hh in range(2 * H_PER_CORE):
                                x = xs[hh]
                                sw = p_sw.tile([P, T], f32, tag="sw")
                                nc.gpsimd.tensor_copy(sw[0:64, :], x[64:P, :])
                                nc.gpsimd.tensor_copy(sw[64:P, :], x[0:64, :])
                                nc.gpsimd.tensor_mul(sw[:], sw[:], s128[:])
                                nc.vector.tensor_mul(x[:], x[:], c128[:])
                                nc.vector.tensor_add(x[:], x[:], sw[:])

                            if dump:
                                for hh in range(2 * H_PER_CORE):
                                    nc.sync.dma_start(d_x[hh], xs[hh][:].bitcast(f32))

                            # attention per q head
                            for h in range(H_PER_CORE):
                                xq = xs[h]
                                xk = xs[4 + h]
                                xv = p_v.tile([P, T], f32r, tag="xv")
                                nc.sync.dma_start(xv[:], qkvT[8 + h, :, :].bitcast(f32r))
                                v_sb = p_vsb.tile([P, T // P, P], f32r, tag="v_sb")
                                for g in range(4):
                                    pv = ps_misc.tile([P, 4, P], f32r, tag="misc")
                                    for j in range(4):
                                        i = g * 4 + j
                                        nc.tensor.transpose(
                                            pv[:, j, :], xv[:, i * P : (i + 1) * P], ident[:]
                                        )
                                    nc.any.tensor_copy(v_sb[:, g * 4 : g * 4 + 4, :], pv[:])
                                with (
                                    tc.tile_pool(name="ps_st2", bufs=2, space="PSUM") as ps_st2,
                                    tc.tile_pool(name="ps_pv", bufs=2, space="PSUM") as ps_pv,
                                ):
                                    for b in range(4):
                                        bs = slice(b * 512, (b + 1) * 512)
                                        at_ps = ps_pv.tile([P, 512], f32, tag="at")
                                        rsum = ps_pv.tile([1, 512], f32, tag="rsum")
                                        n_i_tiles = 4 * b + 4
                                        for i in range(n_i_tiles):
                                            moff = max(0, (i - 4 * b) * P)
                                            nv = 512 - moff
                                            st_ps = ps_st2.tile([P, 512], f32, tag="st2")
                                            nc.tensor.matmul(
                                                st_ps[:, 0:nv],
                                                xk[:, i * P : (i + 1) * P],
                                                xq[:, b * 512 + moff : (b + 1) * 512],
                                                start=True, stop=True)
                                            E = p_E.tile([P, 512], f32r, tag="E")
                                            if moff:
                                                nc.vector.tensor_copy(
                                                    E[:, 0:moff], zeros_r[:, 0:moff])
                                            nc.scalar.activation(
                                                E[:, moff:512], st_ps[:, 0:nv], AF.Exp,
                                                scale=SCALE)
                                            if i >= 4 * b:
                                                nc.gpsimd.tensor_mul(
                                                    E[:, moff : moff + P],
                                                    E[:, moff : moff + P],
                                                    triu[:])
                                            nc.tensor.matmul(
                                                at_ps[:], v_sb[:, i, :], E[:],
                                                start=(i == 0), stop=(i == n_i_tiles - 1))
                                            nc.tensor.matmul(
                                                rsum[:], ones_c[:], E[:],
                                                start=(i == 0), stop=(i == n_i_tiles - 1))
                                        recip = p_rec.tile([1, 512], f32, tag="recip")
                                        nc.vector.reciprocal(recip[:], rsum[:])
                                        recb = p_rec.tile([P, 512], f32, tag="recb")
                                        nc.gpsimd.partition_broadcast(recb[:], recip[:])
                                        nc.vector.tensor_mul(attnT[:, h, bs], at_ps[:], recb[:])

                    if dump:
                        nc.sync.dma_start(d_attnT[:], attnT[:].bitcast(f32))

                    if "3" in phases:
                        if "2" not in phases:
                            nc.sync.dma_start(attnT[:], attnT_in[:].bitcast(f32r))
                        # ---------------- P3: o_proj ----------------
                        with (
                            tc.tile_pool(name="p3_wld", bufs=2) as p3_wld,
                            tc.tile_pool(name="p3_wT", bufs=2) as p3_wT,
                            tc.tile_pool(name="p3_o", bufs=3) as p3_o,
                            tc.tile_pool(name="ps_o", bufs=2, space="PSUM") as ps_o,
                        ):
                            for nb in range(HID // 512):
                                wold = p3_wld.tile([P, 4, 512], f32r, tag="wold")
                                nc.sync.dma_start(
                                    wold[:],
                                    wo[nb * 512 : (nb + 1) * 512, :]
                                    .rearrange("(a p) c -> p a c", p=P)
                                    .bitcast(f32r),
                                )
                                woT = p3_wT.tile([P, 4, 512], f32r, tag="woT")
                                for c in range(4):
                                    pw = ps_misc.tile([P, 4, P], f32r, tag="misc")
                                    for j in range(4):
                                        nc.tensor.transpose(
                                            pw[:, j, :],
                                            wold[:, j, c * P : (c + 1) * P],
                                            ident[:],
                                        )
                                    nc.any.tensor_copy(
                                        woT[:, c, :],
                                        pw[:].rearrange("p a b -> p (a b)"),
                                    )
                                for tg in range(T // (4 * P)):
                                    o_sb = p3_o.tile([P, 4, 512], f32, tag="o_sb")
                                    for j in range(4):
                                        t = tg * 4 + j
                                        po = ps_o.tile([P, 512], f32, tag="po")
                                        for c in range(4):
                                            nc.tensor.matmul(
                                                po[:],
                                                attnT[:, c, t * P : (t + 1) * P],
                                                woT[:, c, :],
                                                start=(c == 0), stop=(c == 3))
                                        nc.any.tensor_copy(o_sb[:, j, :], po[:])
                                    nc.sync.dma_start(
                                        out[tg * 4 * P : (tg + 1) * 4 * P,
                                            nb * 512 : (nb + 1) * 512]
                                        .rearrange("(a p) n -> p a n", p=P),
                                        o_sb[:])


            if n_iters == 1:
                _phases()
            else:
                with tc.For_i(0, n_iters, 1) as _iv:
                    _phases(_iv)

    nc.compile()
    return nc


def _get_nc(n_iters: int = 1):
    if n_iters not in _NC_CACHE:
        _NC_CACHE[n_iters] = build_nc(n_iters)
    return _NC_CACHE[n_iters]


def _shard_inputs(positions, hidden_states, w_qkv, w_o, q_norm_w, q_norm_b,
                  k_norm_w, k_norm_b):
    H = 32
    in_maps = []
    for c in range(8):
        hs = slice(c * H_PER_CORE, (c + 1) * H_PER_CORE)
        rows = np.concatenate(
            [
                w_qkv[c * 512 : (c + 1) * 512],
                w_qkv[H * D + c * 512 : H * D + (c + 1) * 512],
                w_qkv[2 * H * D + c * 512 : 2 * H * D + (c + 1) * 512],
            ],
            axis=0,
        )
        in_maps.append(
            {
                "hidden": np.ascontiguousarray(hidden_states, dtype=np.float32),
                "wq": np.ascontiguousarray(rows, dtype=np.float32),
                "wo": np.ascontiguousarray(w_o[:, c * 512 : (c + 1) * 512],
                                           dtype=np.float32),
                "pos": np.ascontiguousarray(positions, dtype=np.int32),
                "qnw": np.ascontiguousarray(q_norm_w[hs], dtype=np.float32),
                "qnb": np.ascontiguousarray(q_norm_b[hs], dtype=np.float32),
                "knw": np.ascontiguousarray(k_norm_w[hs], dtype=np.float32),
                "knb": np.ascontiguousarray(k_norm_b[hs], dtype=np.float32),
            }
        )
    return in_maps


LAST_RESULTS = None


def kernel(positions, hidden_states, w_qkv, w_o, q_norm_w, q_norm_b,
           k_norm_w, k_norm_b):
    global LAST_RESULTS
    nc = _get_nc(1)
    in_maps = _shard_inputs(
        np.asarray(positions), np.asarray(hidden_states), np.asarray(w_qkv),
        np.asarray(w_o), np.asarray(q_norm_w), np.asarray(q_norm_b),
        np.asarray(k_norm_w), np.asarray(k_norm_b),
    )
    LAST_RESULTS = run_bass_kernel_spmd(nc, in_maps, list(range(8)))
    res = LAST_RESULTS.results
    acc = np.zeros((T, HID), np.float64)
    for c in range(8):
        acc += res[c]["out"].astype(np.float64)
    return acc.astype(np.float32)


if __name__ == "__main__":
    build_nc(1)
    print("build OK")



# revision 3
# speedup vs baseline: 1.8276x; 1.8276x over previous
"""ChameleonAttention Trainium2 kernel, v2 (bf16 + fused P1/LN/RoPE).

Full-input contract: kernel(**inputs); tensor-parallel across 8 cores by
attention head (4 heads/core): w_qkv rows + norm params sharded by head,
w_o columns sharded by head, bf16 partial outputs summed on host.

Per-core dataflow (all matmuls bf16, 1 cycle/row, FWL weight loads):
  Host precomputes: hiddenT, wqkT, wvT, woT (pre-transposed bf16), RoPE
  cos/sin tables, triu mask, ones, norm params stacked [w;b].
  P1 (per 512-token chunk, LN/RoPE chains staggered 3 steps behind the
  matmul chains so PE never idles):
    qk[d,t] = wqkT.T @ hT   (PSUM chain over 32 k-tiles)
    LN stats via ones-matmul (s1,s2) -> scalar chain with
    rstd = exp(-0.5*ln(var+eps)) (no banned Rsqrt, no slow DVE recip);
    LN applied as x*A+B with rank-1 A = w (x) rstd, B = w (x) (-mu*rstd) + b
    built by K=1/K=2 PE broadcast matmuls; neox RoPE with the half-swap
    done by SBUF->SBUF DMA and the multiplies on DVE in bf16 (4x mode).
    v[t,d] = hT_tile.T @ wvT directly in PV layout (no PE transposes).
  P2 attention per head, S^T streamed over key tiles: E = exp(scale*S^T)
  out of PSUM in bf16, causal diag masked by DVE triu multiply, P@V and
  denominator (ones-matmul) accumulated in PSUM; 1/sum via exp(-ln(sum));
  broadcast by K=1 PE matmul.
  P3 o_proj: out[t,:] = attnT.T @ woT, bf16 partials to HBM.
"""
import sys

sys.path.insert(0, "/opt/trn_rl_repo")

import numpy as np
from ml_dtypes import bfloat16

import concourse.bass as bass
import concourse.mybir as mybir
import concourse.tile as tile
from concourse import bacc
from concourse.bass_utils import run_bass_kernel_spmd

P = 128
T = 2048
HID = 4096
D = 128
H = 32
HPC = 4            # heads per core
NK = HID // P      # 32 contraction tiles
TC = 512           # P1 token chunk
NCH = T // TC      # 4 chunks
NTT = T // P       # 16 token tiles
NB = T // 512      # 4 q-blocks per head in attention
THETA = 10000.0
EPS = 1e-5
SCALE = D ** -0.5

f32 = mybir.dt.float32
bf16 = mybir.dt.bfloat16
AF = mybir.ActivationFunctionType
ALU = mybir.AluOpType

_NC_CACHE = {}


def build_nc():
    nc = bacc.Bacc(None, target_bir_lowering=False, debug=False)

    hT = nc.dram_tensor("hT", (HID, T), bf16, kind="ExternalInput")
    wqkT = nc.dram_tensor("wqkT", (HID, 2 * HPC * P), bf16, kind="ExternalInput")
    wvT = nc.dram_tensor("wvT", (HID, HPC * P), bf16, kind="ExternalInput")
    woT = nc.dram_tensor("woT", (HPC * P, HID), bf16, kind="ExternalInput")
    ctab = nc.dram_tensor("ctab", (P, T), bf16, kind="ExternalInput")
    stab = nc.dram_tensor("stab", (P, T), bf16, kind="ExternalInput")
    nprm = nc.dram_tensor("nprm", (2, 2 * HPC, P), bf16, kind="ExternalInput")
    trim = nc.dram_tensor("trim", (P, P), bf16, kind="ExternalInput")
    out = nc.dram_tensor("out", (T, HID), bf16, kind="ExternalOutput")

    with tile.TileContext(nc) as tc:
        with tc.tile_pool(name="state", bufs=1) as state:
            ctab_s = state.tile([P, T], bf16, name="ctab_s")
            nc.sync.dma_start(ctab_s[:], ctab[:, :])
            stab_s = state.tile([P, T], bf16, name="stab_s")
            nc.sync.dma_start(stab_s[:], stab[:, :])
            nprm_w = state.tile([1, 2 * HPC, P], bf16, name="nprm_w")
            nc.sync.dma_start(nprm_w[:], nprm[0:1, :, :])
            nprm_b = state.tile([1, 2 * HPC, P], bf16, name="nprm_b")
            nc.sync.dma_start(nprm_b[:], nprm[1:2, :, :])
            triu_s = state.tile([P, P], bf16, name="triu_s")
            nc.sync.dma_start(triu_s[:], trim[:, :])
            ones_c = state.tile([P, 1], bf16, name="ones_c")
            nc.vector.memset(ones_c[:], 1.0)
            ones_r = state.tile([1, P], bf16, name="ones_r")
            nc.vector.memset(ones_r[:], 1.0)
            ones_tc = state.tile([1, TC], bf16, name="ones_tc")
            nc.vector.memset(ones_tc[:], 1.0)
            eps_t = state.tile([1, 1], f32, name="eps_t")
            nc.vector.memset(eps_t[:], EPS)
            zero_t = state.tile([1, 1], f32, name="zero_t")
            nc.vector.memset(zero_t[:], 0.0)

            xqk = [state.tile([P, T], bf16, name=f"xqk{i}") for i in range(8)]
            v_sb = state.tile([P, NTT, HPC * P], bf16, name="v_sb")
            attnT = state.tile([P, HPC, T], bf16, name="attnT")

            # ---------------- P1: qkv projection + LN + RoPE ----------------
            with (
                tc.tile_pool(name="p1_h", bufs=2) as p_h,
                tc.tile_pool(name="p1_w", bufs=2) as p_w,
                tc.tile_pool(name="p1_wv", bufs=1) as p_wv,
                tc.tile_pool(name="p1_xr", bufs=2) as p_xr,
                tc.tile_pool(name="p1_sm", bufs=1) as p_sm,
                tc.tile_pool(name="p1_nm", bufs=1) as p_nm,
                tc.tile_pool(name="p1_xl", bufs=2) as p_xl,
                tc.tile_pool(name="ps_qk", bufs=2, space="PSUM") as ps_qk,
                tc.tile_pool(name="ps_st", bufs=1, space="PSUM") as ps_st,
                tc.tile_pool(name="ps_ab", bufs=1, space="PSUM") as ps_ab,
                tc.tile_pool(name="ps_v", bufs=2, space="PSUM") as ps_v,
            ):
                wvT_st = p_wv.tile([P, NK, HPC * P], bf16, name="wvT_st")
                nc.sync.dma_start(
                    wvT_st[:], wvT[:, :].rearrange("(a p) m -> p a m", p=P)
                )

                hts = {}
                wsts = {}

                def load_ht(ch):
                    t = p_h.tile([P, NK, TC], bf16, tag="hT")
                    nc.sync.dma_start(
                        t[:],
                        hT[:, ch * TC : (ch + 1) * TC].rearrange(
                            "(a p) t -> p a t", p=P
                        ),
                    )
                    hts[ch] = t

                def load_w(rt):
                    t = p_w.tile([P, NK, P], bf16, tag="wst")
                    nc.sync.dma_start(
                        t[:],
                        wqkT[:, rt * P : (rt + 1) * P].rearrange(
                            "(a p) m -> p a m", p=P
                        ),
                    )
                    wsts[rt] = t

                load_ht(0)
                for ch in range(NCH):
                    if ch + 1 < NCH:
                        load_ht(ch + 1)
                    ht = hts[ch]
                    cs = slice(ch * TC, (ch + 1) * TC)
                    load_w(0)
                    pend = {}
                    for s in range(12):
                        if s < 8:
                            if s + 1 < 8:
                                load_w(s + 1)
                            rt = s
                            ps = ps_qk.tile([P, TC], f32, tag="qkps")
                            w = wsts.pop(rt)
                            for kk in range(NK):
                                nc.tensor.matmul(
                                    ps[:],
                                    w[:, kk, :],
                                    ht[:, kk, :],
                                    start=(kk == 0),
                                    stop=(kk == NK - 1),
                                )
                            xr = p_xr.tile([P, TC], bf16, tag="xr")
                            nc.scalar.activation(xr[:], ps[:], AF.Copy)
                            sq = p_xr.tile([P, TC], bf16, tag="sq")
                            nc.scalar.activation(sq[:], xr[:], AF.Square)
                            pend[rt] = {"xr": xr, "sq": sq}
                        else:
                            ti = s - 8
                            tt = ch * 4 + ti
                            vps = ps_v.tile([P, HPC * P], f32, tag="vps")
                            for kk in range(NK):
                                nc.tensor.matmul(
                                    vps[:],
                                    ht[:, kk, ti * P : (ti + 1) * P],
                                    wvT_st[:, kk, :],
                                    start=(kk == 0),
                                    stop=(kk == NK - 1),
                                )
                            nc.scalar.activation(v_sb[:, tt, :], vps[:], AF.Copy)

                        j = s - 1
                        if 0 <= j < 8:
                            d = pend[j]
                            s1 = ps_st.tile([1, TC], f32, tag="s1")
                            nc.tensor.matmul(
                                s1[:], ones_c[:], d["xr"][:], start=True, stop=True
                            )
                            s2 = ps_st.tile([1, TC], f32, tag="s2")
                            nc.tensor.matmul(
                                s2[:], ones_c[:], d["sq"][:], start=True, stop=True
                            )
                            d["s1"], d["s2"] = s1, s2

                        j = s - 2
                        if 0 <= j < 8:
                            d = pend[j]
                            mu = p_sm.tile([1, TC], bf16, tag="mu")
                            nc.scalar.activation(
                                mu[:], d["s1"][:], AF.Copy, scale=1.0 / D
                            )
                            musq = p_sm.tile([1, TC], bf16, tag="musq")
                            nc.scalar.activation(musq[:], mu[:], AF.Square)
                            s2d = p_sm.tile([1, TC], bf16, tag="s2d")
                            nc.scalar.activation(
                                s2d[:], d["s2"][:], AF.Copy, scale=1.0 / D
                            )
                            varv = p_sm.tile([1, TC], bf16, tag="varv")
                            nc.vector.tensor_sub(varv[:], s2d[:], musq[:])
                            lnv = p_sm.tile([1, TC], bf16, tag="lnv")
                            nc.scalar.activation(
                                lnv[:], varv[:], AF.Ln, bias=eps_t[:]
                            )
                            rstd = p_sm.tile([1, TC], bf16, tag="rstd")
                            nc.scalar.activation(rstd[:], lnv[:], AF.Exp, scale=-0.5)
                            nm = p_nm.tile([1, TC], bf16, tag="nm")
                            nc.vector.scalar_tensor_tensor(
                                out=nm[:],
                                in0=mu[:],
                                scalar=-1.0,
                                in1=rstd[:],
                                op0=ALU.mult,
                                op1=ALU.mult,
                            )
                            A = ps_ab.tile([P, TC], f32, tag="A")
                            nc.tensor.matmul(
                                A[:], nprm_w[0:1, j, :], rstd[:], start=True, stop=True
                            )
                            B = ps_ab.tile([P, TC], f32, tag="B")
                            nc.tensor.matmul(
                                B[:], nprm_w[0:1, j, :], nm[:], start=True, stop=False
                            )
                            nc.tensor.matmul(
                                B[:], nprm_b[0:1, j, :], ones_tc[:], start=False, stop=True
                            )
                            xl = p_xl.tile([P, TC], bf16, tag="xl")
                            nc.vector.tensor_mul(xl[:], d["xr"][:], A[:])
                            nc.vector.tensor_add(xl[:], xl[:], B[:])
                            d["xl"] = xl

                        j = s - 3
                        if 0 <= j < 8:
                            d = pend.pop(j)
                            xl = d["xl"]
                            sw = p_xl.tile([P, TC], bf16, tag="sw")
                            nc.sync.dma_start(sw[0:64, :], xl[64:P, :])
                            nc.sync.dma_start(sw[64:P, :], xl[0:64, :])
                            t1 = p_xl.tile([P, TC], bf16, tag="t1")
                            nc.vector.tensor_mul(t1[:], xl[:], ctab_s[:, cs])
                            nc.vector.tensor_mul(sw[:], sw[:], stab_s[:, cs])
                            nc.vector.tensor_add(xqk[j][:, cs], t1[:], sw[:])

            # ---------------- P2: attention + P3: o_proj ----------------
            with (
                tc.tile_pool(name="p2_wo", bufs=1) as p_wo,
                tc.tile_pool(name="p2_E", bufs=3) as p_E,
                tc.tile_pool(name="p2_sm", bufs=2) as p2_sm,
                tc.tile_pool(name="ps_s", bufs=3, space="PSUM") as ps_s,
                tc.tile_pool(name="ps_at", bufs=2, space="PSUM") as ps_at,
                tc.tile_pool(name="ps_rs", bufs=2, space="PSUM") as ps_rs,
            ):
                woT_st = p_wo.tile([P, HPC, HID], bf16, name="woT_st")
                nc.sync.dma_start(
                    woT_st[:], woT[:, :].rearrange("(c p) n -> p c n", p=P)
                )

                for h in range(HPC):
                    xq = xqk[h]
                    xk = xqk[HPC + h]
                    for b in range(NB):
                        n_i = 4 * b + 4
                        at = ps_at.tile([P, 512], f32, tag="at")
                        rs = ps_rs.tile([1, 512], f32, tag="rs")
                        prev = None

                        def flush(i):
                            nc.tensor.matmul(
                                at[:],
                                v_sb[:, i, h * P : (h + 1) * P],
                                Es[i][:],
                                start=(i == 0),
                                stop=(i == n_i - 1),
                            )
                            nc.tensor.matmul(
                                rs[:],
                                ones_c[:],
                                Es[i][:],
                                start=(i == 0),
                                stop=(i == n_i - 1),
                            )

                        Es = {}
                        for i in range(n_i):
                            moff = max(0, (i - 4 * b) * P)
                            nv = 512 - moff
                            st = ps_s.tile([P, 512], f32, tag="st")
                            nc.tensor.matmul(
                                st[:, 0:nv],
                                xk[:, i * P : (i + 1) * P],
                                xq[:, b * 512 + moff : (b + 1) * 512],
                                start=True,
                                stop=True,
                            )
                            E = p_E.tile([P, 512], bf16, tag="E")
                            Es[i] = E
                            if moff:
                                nc.vector.memset(E[:, 0:moff], 0.0)
                            nc.scalar.activation(
                                E[:, moff:512], st[:, 0:nv], AF.Exp, scale=SCALE
                            )
                            if i >= 4 * b:
                                nc.vector.tensor_mul(
                                    E[:, moff : moff + P],
                                    E[:, moff : moff + P],
                                    triu_s[:],
                                )
                            if prev is not None:
                                flush(prev)
                            prev = i
                        flush(prev)

                        lnr = p2_sm.tile([1, 512], f32, tag="lnr")
                        nc.scalar.activation(lnr[:], rs[:], AF.Ln, bias=zero_t[:])
                        rcp = p2_sm.tile([1, 512], bf16, tag="rcp")
                        nc.scalar.activation(rcp[:], lnr[:], AF.Exp, scale=-1.0)
                        rb = ps_s.tile([P, 512], f32, tag="st")
                        nc.tensor.matmul(rb[:], ones_r[:], rcp[:], start=True, stop=True)
                        rbs = p2_sm.tile([P, 512], bf16, tag="rbs")
                        nc.scalar.activation(rbs[:], rb[:], AF.Copy)
                        nc.vector.tensor_mul(
                            attnT[:, h, b * 512 : (b + 1) * 512], at[:], rbs[:]
                        )

                # ---------------- P3 ----------------
                with (
                    tc.tile_pool(name="p3_o", bufs=2) as p3_o,
                    tc.tile_pool(name="ps_po", bufs=2, space="PSUM") as ps_po,
                ):
                    for t in range(NTT):
                        for half in range(2):
                            po = ps_po.tile([P, 4, 512], f32, tag="po")
                            for c in range(HPC):
                                for n4 in range(4):
                                    nc.tensor.matmul(
                                        po[:, n4, :],
                                        attnT[:, c, t * P : (t + 1) * P],
                                        woT_st[
                                            :,
                                            c,
                                            half * 2048
                                            + n4 * 512 : half * 2048
                                            + (n4 + 1) * 512,
                                        ],
                                        start=(c == 0),
                                        stop=(c == HPC - 1),
                                    )
                            ost = p3_o.tile([P, 4, 512], bf16, tag="ost")
                            if t % 2 == 0:
                                nc.scalar.activation(ost[:], po[:], AF.Copy)
                            else:
                                nc.vector.tensor_copy(ost[:], po[:])
                            nc.sync.dma_start(
                                out[
                                    t * P : (t + 1) * P,
                                    half * 2048 : (half + 1) * 2048,
                                ],
                                ost[:].rearrange("p a n -> p (a n)"),
                            )

    nc.compile()
    return nc


def _get_nc():
    if "nc" not in _NC_CACHE:
        _NC_CACHE["nc"] = build_nc()
    return _NC_CACHE["nc"]


def _make_in_maps(positions, hidden_states, w_qkv, w_o, q_norm_w, q_norm_b,
                  k_norm_w, k_norm_b):
    bf = bfloat16
    hidT = np.ascontiguousarray(hidden_states.astype(np.float32).T).astype(bf)

    pos = positions.astype(np.float32)
    inv = THETA ** (-np.arange(64, dtype=np.float32) / 64.0)
    fr = pos[None, :] * inv[:, None]  # [64, T]
    cosv = np.cos(fr)
    sinv = np.sin(fr)
    ctab_np = np.concatenate([cosv, cosv], axis=0).astype(bf)
    stab_np = np.concatenate([-sinv, sinv], axis=0).astype(bf)

    triu_np = np.triu(np.ones((P, P), dtype=np.float32)).astype(bf)

    in_maps = []
    for c in range(8):
        hs = slice(c * HPC, (c + 1) * HPC)
        wq_c = w_qkv[c * 512 : (c + 1) * 512]
        wk_c = w_qkv[H * D + c * 512 : H * D + (c + 1) * 512]
        wv_c = w_qkv[2 * H * D + c * 512 : 2 * H * D + (c + 1) * 512]
        wqkT_np = np.ascontiguousarray(
            np.concatenate([wq_c, wk_c], axis=0).astype(np.float32).T
        ).astype(bf)
        wvT_np = np.ascontiguousarray(wv_c.astype(np.float32).T).astype(bf)
        woT_np = np.ascontiguousarray(
            w_o[:, c * 512 : (c + 1) * 512].astype(np.float32).T
        ).astype(bf)
        nprm_np = np.stack(
            [
                np.concatenate([q_norm_w[hs], k_norm_w[hs]], axis=0),
                np.concatenate([q_norm_b[hs], k_norm_b[hs]], axis=0),
            ],
            axis=0,
        ).astype(bf)  # [2, 8, 128]
        in_maps.append(
            {
                "hT": hidT,
                "wqkT": wqkT_np,
                "wvT": wvT_np,
                "woT": woT_np,
                "ctab": ctab_np,
                "stab": stab_np,
                "nprm": nprm_np,
                "trim": triu_np,
            }
        )
    return in_maps


LAST_RESULTS = None


def kernel(positions, hidden_states, w_qkv, w_o, q_norm_w, q_norm_b,
           k_norm_w, k_norm_b):
    global LAST_RESULTS
    nc = _get_nc()
    in_maps = _make_in_maps(
        np.asarray(positions), np.asarray(hidden_states), np.asarray(w_qkv),
        np.asarray(w_o), np.asarray(q_norm_w), np.asarray(q_norm_b),
        np.asarray(k_norm_w), np.asarray(k_norm_b),
    )
    LAST_RESULTS = run_bass_kernel_spmd(nc, in_maps, list(range(8)))
    res = LAST_RESULTS.results
    acc = np.zeros((T, HID), np.float32)
    for c in range(8):
        acc += np.asarray(res[c]["out"]).astype(np.float32)
    return acc


if __name__ == "__main__":
    build_nc()
    print("build OK")
